# revision 48
# baseline (speedup 1.0000x reference)
"""Multi-head attention (B=4, S=2048, D=1024, H=16) + residual + LayerNorm
on 8 Trainium2 NeuronCores.  (verified on HW: 707 us, rel-err 4.7e-5)

Sharding: 8 independent cores, no collectives. Core i handles batch b=i//2
and query rows [(i%2)*1024, (i%2)*1024+1024), all 16 heads.
"""

import sys

for _p in ("/opt/trn_rl_repo", "/root/.axon_site/_ro/trn_rl_repo"):
    if _p not in sys.path:
        sys.path.insert(0, _p)

import numpy as np
import ml_dtypes

D = 1024
H = 16
DK = 64
B = 4
S = 2048
QR = 1024
P = 128
N_CORES = 8

BF16 = ml_dtypes.bfloat16

_CACHED = {}


def _build_bass():
    import concourse.bass as bass
    import concourse.tile as tile
    from concourse import bacc, mybir

    f32 = mybir.dt.float32
    bf16 = mybir.dt.bfloat16
    Exp = mybir.ActivationFunctionType.Exp
    Sqrt = mybir.ActivationFunctionType.Sqrt
    Identity = mybir.ActivationFunctionType.Identity
    Alu = mybir.AluOpType

    nc = bacc.Bacc("TRN2", target_bir_lowering=False)

    qT_d = nc.declare_dram_parameter("qT", [D, QR], bf16, isOutput=False)
    kT_d = nc.declare_dram_parameter("kT", [D, S], bf16, isOutput=False)
    vT_d = nc.declare_dram_parameter("vT", [D, S], bf16, isOutput=False)
    qnat_d = nc.declare_dram_parameter("q_nat", [QR, D], f32, isOutput=False)
    Wq_d = nc.declare_dram_parameter("Wq", [D, D], bf16, isOutput=False)
    Wk_d = nc.declare_dram_parameter("Wk", [D, D], bf16, isOutput=False)
    Wv_d = nc.declare_dram_parameter("Wv", [D, D], bf16, isOutput=False)
    Wo_d = nc.declare_dram_parameter("Wo_bf", [D, D], bf16, isOutput=False)
    bq_d = nc.declare_dram_parameter("bq_p", [P, D // P], f32, isOutput=False)
    bk_d = nc.declare_dram_parameter("bk_p", [P, D // P], f32, isOutput=False)
    bv_d = nc.declare_dram_parameter("bv_r", [1, D], f32, isOutput=False)
    bo_d = nc.declare_dram_parameter("bo_r", [1, D], f32, isOutput=False)
    gamma_d = nc.declare_dram_parameter("gamma_r", [1, D], f32, isOutput=False)
    beta_d = nc.declare_dram_parameter("beta_r", [1, D], f32, isOutput=False)
    out_d = nc.declare_dram_parameter("out", [QR, D], f32, isOutput=True)

    DT = D // P
    ST = S // P
    NQC = QR // 512

    def bcast_ap(handle, parts):
        ap = handle[:, :]
        return bass.AP(
            tensor=ap.tensor, offset=ap.offset, ap=[[0, parts]] + list(ap.ap[1:])
        )

    with tile.TileContext(nc) as tc:
        with tc.tile_pool(name="persist", bufs=1) as persist:
            V_sb = persist.tile([P, ST, H, DK + 1], bf16)
            ctxT_sb = persist.tile([P, DT, QR], bf16)
            bv_b = persist.tile([P, D], f32)
            bq_sb = persist.tile([P, DT], f32)
            bk_sb = persist.tile([P, DT], f32)
            ones_sb = persist.tile([1, DK], bf16)
            eps_sb = persist.tile([P, 1], f32)

            nc.sync.dma_start(out=bv_b, in_=bcast_ap(bv_d, P))
            nc.sync.dma_start(out=bq_sb, in_=bq_d[:, :])
            nc.sync.dma_start(out=bk_sb, in_=bk_d[:, :])
            nc.vector.memset(ones_sb, 1.0)
            nc.vector.memset(eps_sb, 1e-5)
            for st in range(ST):
                nc.vector.memset(V_sb[:, st, :, DK : DK + 1], 1.0)

            with (
                tc.tile_pool(name="wqkpool", bufs=2) as wqkpool,
                tc.tile_pool(name="wvpool", bufs=1) as wvpool,
                tc.tile_pool(name="xpool", bufs=2) as xpool,
                tc.tile_pool(name="qkpool", bufs=2) as qkpool,
                tc.tile_pool(name="epool", bufs=2) as epool,
                tc.tile_pool(name="rpool", bufs=2) as rpool,
                tc.tile_pool(name="rbpool", bufs=2) as rbpool,
                tc.tile_pool(name="spsum", bufs=2, space="PSUM") as spsum,
                tc.tile_pool(name="avpsum", bufs=2, space="PSUM") as avpsum,
            ):

                def x_dma(x_dram, sc):
                    x_sb = xpool.tile([P, DT, 512], bf16, tag="x")
                    for kt in range(DT):
                        nc.sync.dma_start(
                            out=x_sb[:, kt, :],
                            in_=x_dram[
                                kt * P : (kt + 1) * P, sc * 512 : (sc + 1) * 512
                            ],
                        )
                    return x_sb

                def proj_mms(W_sb, wcol, x_sb, dt, out_t, bias_sb, sc):
                    # out_t[:, sc-chunk] (bf16) = (W.T @ x)[dt-tile, .] + bias
                    ps = spsum.tile([P, 512], f32, tag="ps_s")
                    for kt in range(DT):
                        nc.tensor.matmul(
                            ps,
                            lhsT=W_sb[:, kt, wcol * P : (wcol + 1) * P],
                            rhs=x_sb[:, kt, :],
                            start=(kt == 0),
                            stop=(kt == DT - 1),
                        )
                    nc.vector.tensor_scalar_add(
                        out=out_t[:, sc * 512 : (sc + 1) * 512],
                        in0=ps,
                        scalar1=bias_sb[:, dt : dt + 1],
                    )

                def load_wqk(p):
                    Wqk_sb = wqkpool.tile([P, DT, 2 * P], bf16, tag="wqk")
                    for kt in range(DT):
                        nc.sync.dma_start(
                            out=Wqk_sb[:, kt, 0:P],
                            in_=Wq_d[kt * P : (kt + 1) * P, p * P : (p + 1) * P],
                        )
                        nc.sync.dma_start(
                            out=Wqk_sb[:, kt, P : 2 * P],
                            in_=Wk_d[kt * P : (kt + 1) * P, p * P : (p + 1) * P],
                        )
                    return Wqk_sb

                def qk_proj_groups(p, Wqk_sb):
                    # yields after each projection psum-group of d-tile p,
                    # with the next group's x-chunk DMAs prefetched one slot
                    # ahead; the last yield carries the (qt_t, kt_t) tiles
                    qt_t = qkpool.tile([P, QR], bf16, tag="qt")
                    kt_t = qkpool.tile([P, S], bf16, tag="kt")
                    specs = [(0, qT_d, qt_t, bq_sb, sc) for sc in range(QR // 512)]
                    specs += [(1, kT_d, kt_t, bk_sb, sc) for sc in range(S // 512)]
                    x_next = x_dma(specs[0][1], specs[0][4])
                    for i, (wcol, xd, out_t, bias, sc) in enumerate(specs):
                        x_sb = x_next
                        if i + 1 < len(specs):
                            x_next = x_dma(specs[i + 1][1], specs[i + 1][4])
                        proj_mms(Wqk_sb, wcol, x_sb, p, out_t, bias, sc)
                        yield (qt_t, kt_t) if i == len(specs) - 1 else None

                def v_groups(hc):
                    # generator: V[:, heads 8hc..8hc+8] natural layout, one
                    # psum-group (one s-tile) per yield — 16 yields
                    Wv_sb = wvpool.tile([P, DT, 512], bf16, tag="wv")
                    for kt in range(DT):
                        nc.sync.dma_start(
                            out=Wv_sb[:, kt, :],
                            in_=Wv_d[
                                kt * P : (kt + 1) * P, hc * 512 : (hc + 1) * 512
                            ],
                        )
                    for vc in range(S // 512):
                        x_sb = xpool.tile([P, DT, 512], bf16, tag="x")
                        for kt in range(DT):
                            nc.sync.dma_start(
                                out=x_sb[:, kt, :],
                                in_=vT_d[
                                    kt * P : (kt + 1) * P, vc * 512 : (vc + 1) * 512
                                ],
                            )
                        for sj in range(4):
                            st = vc * 4 + sj
                            ps = spsum.tile([P, 512], f32, tag="ps_s")
                            for kt in range(DT):
                                nc.tensor.matmul(
                                    ps,
                                    lhsT=x_sb[:, kt, sj * P : (sj + 1) * P],
                                    rhs=Wv_sb[:, kt, :],
                                    start=(kt == 0),
                                    stop=(kt == DT - 1),
                                )
                            nc.vector.tensor_tensor(
                                out=V_sb[:, st, hc * 8 : (hc + 1) * 8, 0:DK],
                                in0=ps.rearrange("p (h d) -> p h d", h=8),
                                in1=bv_b[:, hc * 512 : (hc + 1) * 512].rearrange(
                                    "p (h d) -> p h d", h=8
                                ),
                                op=Alu.add,
                            )
                            yield None
                # Deferred normalization: each head's 1/Z reciprocal (a slow
                # single-partition DVE op) is issued right after its A@V
                # accumulation, but the PE-side broadcast + multiply are
                # emitted in the middle of the NEXT head's S-loop so the PE
                # (strict FIFO) never waits on the reciprocal.
                def norm_tail(ps_av, r_bf, dt, po):
                    ps_rb = spsum.tile([DK, QR], f32, tag="ps_s")
                    for qc in range(NQC):
                        nc.tensor.matmul(
                            ps_rb[:, qc * 512 : (qc + 1) * 512],
                            lhsT=ones_sb,
                            rhs=r_bf[:, qc * 512 : (qc + 1) * 512],
                            start=True,
                            stop=True,
                        )
                    r_bc = rbpool.tile([DK, QR], f32, tag="rbc")
                    nc.vector.tensor_copy(out=r_bc, in_=ps_rb)
                    nc.vector.tensor_tensor(
                        out=ctxT_sb[po : po + DK, dt, :],
                        in0=ps_av[0:DK, :],
                        in1=r_bc,
                        op=Alu.mult,
                    )

                # Software pipeline across heads: during head h's S-loop
                # (which is ACT/exp-paced and leaves the PE half idle), the
                # PE also runs head h-1's A@V accumulation. This keeps both
                # engines busy continuously and the PE dense enough to stay
                # at the warm HAM clock.
                # prologue: projections for d-tile 0
                next_qk = [x for x in qk_proj_groups(0, load_wqk(0)) if x][0]

                pending_norm = None
                prev = None  # (E_sb, ps_av, h) of the previous head
                gnext = iter(())
                vg = None
                for h in range(H + 1):
                    if h < H:
                        dt = h // 2
                        po = (h % 2) * DK
                        if h % 2 == 0:
                            qt_t, kt_t = next_qk
                            if dt + 1 < DT:
                                gnext = qk_proj_groups(dt + 1, load_wqk(dt + 1))
                            else:
                                gnext = iter(())
                        if h in (0, 8):
                            vg = v_groups(h // 8)
                        E_sb = epool.tile([P, ST, QR], bf16, tag="E")
                    for t in range(ST):
                        if t == 12 and pending_norm is not None:
                            norm_tail(*pending_norm)
                            pending_norm = None
                        if h < H and vg is not None:
                            # one V-projection psum-group per t in heads 0/8
                            if next(vg, "end") == "end":
                                vg = None
                        if (
                            h < H
                            and h % 2 == 1
                            and t in (1, 3, 5, 9, 11, 13)
                        ):
                            # sprinkle one next-d-tile Q/K projection group
                            r = next(gnext, None)
                            if isinstance(r, tuple):
                                next_qk = r
                        if h < H:
                            ps_s = spsum.tile([P, QR], f32, tag="ps_s")
                            for qc in range(NQC):
                                nc.tensor.matmul(
                                    ps_s[:, qc * 512 : (qc + 1) * 512],
                                    lhsT=kt_t[po : po + DK, t * P : (t + 1) * P],
                                    rhs=qt_t[
                                        po : po + DK, qc * 512 : (qc + 1) * 512
                                    ],
                                    start=True,
                                    stop=True,
                                )
                            nc.scalar.activation(
                                out=E_sb[:, t, :], in_=ps_s, func=Exp, scale=0.125
                            )
                        if prev is not None:
                            E_prev, pa_prev, hp = prev
                            for qc in range(NQC):
                                nc.tensor.matmul(
                                    pa_prev[:, qc * 512 : (qc + 1) * 512],
                                    lhsT=V_sb[:, t, hp, :],
                                    rhs=E_prev[:, t, qc * 512 : (qc + 1) * 512],
                                    start=(t == 0),
                                    stop=(t == ST - 1),
                                )
                    if h < H and h % 2 == 1:
                        # drain any unsprinkled projection groups
                        for r in gnext:
                            if isinstance(r, tuple):
                                next_qk = r
                    if prev is not None:
                        E_prev, pa_prev, hp = prev
                        r_sb = rpool.tile([1, QR], f32, tag="r")
                        nc.vector.reciprocal(out=r_sb, in_=pa_prev[DK : DK + 1, :])
                        r_bf = rpool.tile([1, QR], bf16, tag="rbf")
                        nc.vector.tensor_copy(out=r_bf, in_=r_sb)
                        pending_norm = (pa_prev, r_bf, hp // 2, (hp % 2) * DK)
                    if h < H:
                        ps_av = avpsum.tile([DK + 1, QR], f32, tag="ps_av")
                        prev = (E_sb, ps_av, h)
                norm_tail(*pending_norm)

            with (
                tc.tile_pool(name="wopool", bufs=1) as wopool,
                tc.tile_pool(name="qnpool", bufs=2) as qnpool,
                tc.tile_pool(name="ypool", bufs=2) as ypool,
                tc.tile_pool(name="stpool", bufs=2) as stpool,
                tc.tile_pool(name="opsum", bufs=2, space="PSUM") as opsum,
            ):
                Wo_sb = wopool.tile([P, DT, D], bf16)
                for kt in range(DT):
                    nc.sync.dma_start(
                        out=Wo_sb[:, kt, :], in_=Wo_d[kt * P : (kt + 1) * P, :]
                    )
                bo_b = wopool.tile([P, D], f32)
                gamma_b = wopool.tile([P, D], f32)
                beta_b = wopool.tile([P, D], f32)
                nc.sync.dma_start(out=bo_b, in_=bcast_ap(bo_d, P))
                nc.sync.dma_start(out=gamma_b, in_=bcast_ap(gamma_d, P))
                nc.sync.dma_start(out=beta_b, in_=bcast_ap(beta_d, P))
                for qt in range(QR // P):
                    ps_o = opsum.tile([P, D], f32, tag="ps_o")
                    for oc in range(2):
                        for dct in range(DT):
                            nc.tensor.matmul(
                                ps_o[:, oc * 512 : (oc + 1) * 512],
                                lhsT=ctxT_sb[:, dct, qt * P : (qt + 1) * P],
                                rhs=Wo_sb[:, dct, oc * 512 : (oc + 1) * 512],
                                start=(dct == 0),
                                stop=(dct == DT - 1),
                            )
                    qn = qnpool.tile([P, D], f32, tag="qn")
                    nc.sync.dma_start(out=qn, in_=qnat_d[qt * P : (qt + 1) * P, :])
                    x_sb = ypool.tile([P, D], f32, tag="x")
                    nc.vector.tensor_tensor(out=x_sb, in0=ps_o, in1=qn, op=Alu.add)
                    nc.vector.tensor_tensor(out=x_sb, in0=x_sb, in1=bo_b, op=Alu.add)
                    stats = stpool.tile([P, 2, 6], f32, tag="stats")
                    for g in range(2):
                        nc.vector.bn_stats(
                            out=stats[:, g, :], in_=x_sb[:, g * 512 : (g + 1) * 512]
                        )
                    mv = stpool.tile([P, 2], f32, tag="mv")
                    nc.vector.bn_aggr(out=mv, in_=stats)
                    rstd = stpool.tile([P, 1], f32, tag="rstd")
                    nc.scalar.activation(
                        out=rstd, in_=mv[:, 1:2], func=Sqrt, bias=eps_sb, scale=1.0
                    )
                    nc.vector.reciprocal(out=rstd, in_=rstd)
                    y_sb = ypool.tile([P, D], f32, tag="y")
                    nc.vector.tensor_scalar(
                        out=y_sb,
                        in0=x_sb,
                        scalar1=mv[:, 0:1],
                        scalar2=rstd,
                        op0=Alu.subtract,
                        op1=Alu.mult,
                    )
                    nc.vector.tensor_tensor(out=y_sb, in0=y_sb, in1=gamma_b, op=Alu.mult)
                    nc.vector.tensor_tensor(out=y_sb, in0=y_sb, in1=beta_b, op=Alu.add)
                    nc.sync.dma_start(out=out_d[qt * P : (qt + 1) * P, :], in_=y_sb)

    nc.compile()
    return nc


def _get_nc():
    if "nc" not in _CACHED:
        _CACHED["nc"] = _build_bass()
    return _CACHED["nc"]


def kernel(query, key, value, Wq, bq, Wk, bk, Wv, bv, Wo, bo, gamma, beta):
    from concourse import bass_utils

    query = np.ascontiguousarray(np.asarray(query, np.float32))
    key = np.ascontiguousarray(np.asarray(key, np.float32))
    value = np.ascontiguousarray(np.asarray(value, np.float32))
    Wq = np.ascontiguousarray(np.asarray(Wq, np.float32).astype(BF16))
    Wk = np.ascontiguousarray(np.asarray(Wk, np.float32).astype(BF16))
    Wv = np.ascontiguousarray(np.asarray(Wv, np.float32).astype(BF16))
    Wo_bf = np.ascontiguousarray(np.asarray(Wo, np.float32).astype(BF16))
    bq_p = np.ascontiguousarray(np.asarray(bq, np.float32).reshape(D // P, P).T)
    bk_p = np.ascontiguousarray(np.asarray(bk, np.float32).reshape(D // P, P).T)
    bv_r = np.asarray(bv, np.float32).reshape(1, D)
    bo_r = np.asarray(bo, np.float32).reshape(1, D)
    gamma_r = np.asarray(gamma, np.float32).reshape(1, D)
    beta_r = np.asarray(beta, np.float32).reshape(1, D)

    kT = [np.ascontiguousarray(key[b].T.astype(BF16)) for b in range(B)]
    vT = [np.ascontiguousarray(value[b].T.astype(BF16)) for b in range(B)]

    in_maps = []
    for core in range(N_CORES):
        b, half = core // 2, core % 2
        rows = slice(half * QR, (half + 1) * QR)
        qs = query[b, rows]
        in_maps.append(
            {
                "qT": np.ascontiguousarray(qs.T.astype(BF16)),
                "kT": kT[b],
                "vT": vT[b],
                "q_nat": np.ascontiguousarray(qs),
                "Wq": Wq,
                "Wk": Wk,
                "Wv": Wv,
                "Wo_bf": Wo_bf,
                "bq_p": bq_p,
                "bk_p": bk_p,
                "bv_r": bv_r,
                "bo_r": bo_r,
                "gamma_r": gamma_r,
                "beta_r": beta_r,
            }
        )

    nc = _get_nc()
    res = bass_utils.run_bass_kernel_spmd(nc, in_maps, core_ids=list(range(N_CORES)))
    _CACHED["last_results"] = res

    out = np.empty((B, S, D), np.float32)
    for core in range(N_CORES):
        b, half = core // 2, core % 2
        out[b, half * QR : (half + 1) * QR] = res.results[core]["out"]
    return out


if __name__ == "__main__":
    nc = _get_nc()
    print("bass program built OK")
